# revision 2
# baseline (speedup 1.0000x reference)
"""AttnBlock kernel: GroupNorm + dual-scale (patch/global) attention block.

Contract: kernel(**inputs) takes FULL unsharded inputs (batch B=8) and
returns the FULL output, data-parallel over batch across 8 NeuronCores.
"""

import numpy as np
import jax
import jax.numpy as jnp
from jax.sharding import Mesh, NamedSharding, PartitionSpec as P

B, C, H, W = 8, 256, 112, 112
PATCH = 14
S = (H * W) // (PATCH * PATCH)  # 64
PP = PATCH * PATCH  # 196
TG = PATCH * 4  # 56
A = TG * TG  # 3136
PW, GW = 0.75, 0.25
EPS = 1e-5


def _upsample_mat():
    # 1-D bilinear 2x upsample matrix [H, TG], half-pixel centers.
    x = (np.arange(H, dtype=np.float64) + 0.5) / 2.0 - 0.5
    x0 = np.floor(x).astype(np.int64)
    frac = x - x0
    x0c = np.clip(x0, 0, TG - 1)
    x1c = np.clip(x0 + 1, 0, TG - 1)
    M = np.zeros((H, TG), dtype=np.float64)
    M[np.arange(H), x0c] += 1.0 - frac
    M[np.arange(H), x1c] += frac
    return M.astype(np.float32)


_UP = _upsample_mat()


def _block(x, gn_w, gn_b, wq, bq, wk, bk, wv, bv, w_proj, up):
    # x: [b, C, H, W] (per-shard batch)
    b = x.shape[0]
    mu = jnp.mean(x, axis=(1, 2, 3), keepdims=True)
    var = jnp.mean((x - mu) ** 2, axis=(1, 2, 3), keepdims=True)
    xn = (x - mu) * jax.lax.rsqrt(var + EPS)
    xn = xn * gn_w[None, :, None, None] + gn_b[None, :, None, None]

    xf = xn.reshape(b, C, H * W)
    q = jnp.einsum('oc,bcp->bop', wq, xf) + bq[None, :, None]
    k = jnp.einsum('oc,bcp->bop', wk, xf) + bk[None, :, None]
    v = jnp.einsum('oc,bcp->bop', wv, xf) + bv[None, :, None]

    # ---- patch attention ----
    qm = q.reshape(b, C * S, PP)
    km = k.reshape(b, C * S, PP)
    vm = v.reshape(b, C * S, PP)
    att = jnp.einsum('bdp,bdq->bpq', qm, km) * (C * S) ** -0.5
    att = jax.nn.softmax(att, axis=2)
    h_patch = jnp.einsum('bdp,bqp->bdq', vm, att).reshape(b, C, H * W)

    # ---- global attention on 2x2-avg-pooled maps ----
    def pool(t):
        return t.reshape(b, C, TG, 2, TG, 2).mean(axis=(3, 5)).reshape(b, C, A)

    qg, kg, vg = pool(q), pool(k), pool(v)
    attg = jnp.einsum('bcp,bcq->bpq', qg, kg) * C ** -0.5
    attg = jax.nn.softmax(attg, axis=2)
    hg = jnp.einsum('bcp,bqp->bcq', vg, attg).reshape(b, C, TG, TG)
    # bilinear 2x upsample via separable matmuls
    hu = jnp.einsum('ij,bcjk->bcik', up, hg)
    hu = jnp.einsum('kj,bcij->bcik', up, hu)
    h_glob = hu.reshape(b, C, H * W)

    h = PW * h_patch + GW * h_glob
    out = xf_res = x.reshape(b, C, H * W) + jnp.einsum('oc,bcp->bop', w_proj, h)
    return out.reshape(b, C, H, W)


_jitted = None


def _get_jitted():
    global _jitted
    if _jitted is None:
        devs = jax.devices()[:8]
        mesh = Mesh(np.array(devs), ('b',))
        xs = NamedSharding(mesh, P('b'))
        rs = NamedSharding(mesh, P())
        in_shardings = (xs,) + (rs,) * 10
        _jitted = jax.jit(
            _block,
            in_shardings=in_shardings,
            out_shardings=xs,
        )
    return _jitted


def kernel(x, gn_w, gn_b, wq, bq, wk, bk, wv, bv, w_proj):
    f = _get_jitted()
    out = f(
        jnp.asarray(x, jnp.float32),
        jnp.asarray(gn_w, jnp.float32),
        jnp.asarray(gn_b, jnp.float32),
        jnp.asarray(wq, jnp.float32),
        jnp.asarray(bq, jnp.float32),
        jnp.asarray(wk, jnp.float32),
        jnp.asarray(bk, jnp.float32),
        jnp.asarray(wv, jnp.float32),
        jnp.asarray(bv, jnp.float32),
        jnp.asarray(w_proj, jnp.float32),
        jnp.asarray(_UP),
    )
    return np.asarray(out)
